# revision 7
# baseline (speedup 1.0000x reference)
"""Trainium2 Bass kernel for nn_DNNF (segment_reduce DNF network).

Strategy: data-parallel over batch across 8 NeuronCores (1024 rows each).
The literal axis is host-permuted into 12 phase-planes of 896 columns so the
AND segment-sum (depths cycling [2,4,6]) becomes contiguous vector adds, and
the conjunction axis is ordered group/plane-major so the OR segment-sum is
also contiguous adds. GEMM runs in fp16 on the PE (fp32 PSUM accumulate)
with the tanh applied by the Scalar engine during PSUM eviction.
"""
import numpy as np

import concourse.bacc as bacc
import concourse.mybir as mybir
from concourse import bass_utils
from concourse.tile import TileContext

f32 = mybir.dt.float32
fp16 = mybir.dt.float16
AX = mybir.AxisListType
ALU = mybir.AluOpType
ACTF = mybir.ActivationFunctionType

# problem shape (fixed by the harness)
B, D, L, C, F = 8192, 512, 10752, 2688, 256
NCORES = 8
BS = B // NCORES          # rows per core = 1024
NBT = BS // 128           # b-tiles per core = 8
KT = D // 128             # k-tiles = 4
CW = C // 3               # class width = 896 conj per depth-class
PLANES = L // CW          # 12 literal phase-planes
DEPTHS = (2, 4, 6)
PLANE_BASE = {2: 0, 4: 2, 6: 6}
CLS_OFF = {2: 0, 4: 1, 6: 2}
TEMPERATURE = 2.0

_PROGRAM_CACHE = {}


def _derive_structure(lit2conj, conj2form):
    """Validate the expected DNF structure and return group metadata."""
    depths = np.bincount(lit2conj, minlength=C)
    assert np.array_equal(depths, np.tile(np.array(DEPTHS), C // 3)), \
        "unexpected lit2conj structure"
    cpf = np.bincount(conj2form, minlength=F)
    groups = []          # (formula_start, n_formulas, cpf)
    i = 0
    while i < F:
        j = i
        while j < F and cpf[j] == cpf[i]:
            j += 1
        groups.append((i, j - i, int(cpf[i])))
        i = j
    for (_, nf, c_) in groups:
        assert c_ % 3 == 0, "conj-per-formula not divisible by 3"
    cstart = np.concatenate([[0], np.cumsum(cpf)[:-1]])
    assert np.all(cstart % 3 == 0), "formula conj ranges not 3-aligned"
    return groups, cpf, cstart


def _build_permutation(lit2conj, conj2form, groups, cpf, cstart):
    """Map each literal to its (plane, k) column and conj to class/k index.

    k (0..895) within each depth-class is ordered group-major then
    plane-major then formula-major, which makes both the AND adds
    (literal planes) and the OR adds (conj planes) contiguous.
    """
    conj_depth = np.bincount(lit2conj, minlength=C)
    cls = (np.asarray([CLS_OFF[int(d)] for d in conj_depth]))       # [C]
    # group-class offsets in k-space
    gk0 = {}
    acc = 0
    for gi, (f0, nf, c_) in enumerate(groups):
        gk0[gi] = acc
        acc += nf * (c_ // 3)
    assert acc == CW
    group_of_formula = np.zeros(F, np.int64)
    for gi, (f0, nf, c_) in enumerate(groups):
        group_of_formula[f0:f0 + nf] = gi
    # for each conj: its formula, local formula index, plane j within class
    form_of_conj = np.asarray(conj2form, np.int64)
    g_of_conj = group_of_formula[form_of_conj]
    c3 = np.arange(C) // 3
    s3 = (cstart[form_of_conj] // 3).astype(np.int64)
    j_in_form = c3 - s3                                 # plane within class
    f_local = form_of_conj - np.asarray([groups[g][0] for g in g_of_conj])
    k_of_conj = (np.asarray([gk0[g] for g in g_of_conj])
                 + j_in_form * np.asarray([groups[g][1] for g in g_of_conj])
                 + f_local)
    # literal position within its conj
    first_lit = np.concatenate([[0], np.cumsum(conj_depth)[:-1]])
    lpos = np.arange(L) - first_lit[lit2conj]
    plane = np.asarray([PLANE_BASE[int(d)] for d in conj_depth[lit2conj]]) + lpos
    newcol = plane * CW + k_of_conj[lit2conj]
    assert len(np.unique(newcol)) == L
    inv = np.empty(L, np.int64)
    inv[newcol] = np.arange(L)
    return inv, gk0


def _build_program(groups, gk0, bias_zero):
    key = (tuple(groups), tuple(sorted(gk0.items())), bias_zero)
    if key in _PROGRAM_CACHE:
        return _PROGRAM_CACHE[key]
    assert bias_zero, "nonzero literal bias path not implemented"

    nc = bacc.Bacc("TRN2", target_bir_lowering=False, debug=False,
                   num_devices=NCORES)

    xT_d = nc.dram_tensor("xT", [D, BS], f32, kind="ExternalInput").ap()
    wp_d = nc.dram_tensor("wp", [D, L], f32, kind="ExternalInput").ap()
    mp_d = nc.dram_tensor("mp", [D, L], f32, kind="ExternalInput").ap()
    muT_d = nc.dram_tensor("muT", [D, F], f32, kind="ExternalInput").ap()
    mun_d = nc.dram_tensor("mun", [F, D], f32, kind="ExternalInput").ap()
    sig_d = nc.dram_tensor("sig", [F], f32, kind="ExternalInput").ap()
    eye_d = nc.dram_tensor("eye", [128, 128], fp16, kind="ExternalInput").ap()
    out_d = nc.dram_tensor("out", [BS, F], f32, kind="ExternalOutput").ap()
    scr_d = nc.dram_tensor("m2scr", [F], f32, kind="Internal").ap()

    LN_T = float(np.log(TEMPERATURE))

    with TileContext(nc) as tc:
        with tc.tile_pool(name="cst", bufs=1) as cst, \
             tc.tile_pool(name="stg", bufs=4) as stg, \
             tc.tile_pool(name="wrk", bufs=1) as wrk, \
             tc.tile_pool(name="tail", bufs=1) as tail, \
             tc.tile_pool(name="pp", bufs=2) as ppool, \
             tc.tile_pool(name="ps", bufs=2, space="PSUM") as psp:

            # ---------- constants / prep ----------
            bias_cols = {}

            def bias_col(val):
                v = float(val)
                if v not in bias_cols:
                    t = cst.tile([128, 1], f32, tag=f"bc{len(bias_cols)}")
                    nc.vector.memset(t[:], v)
                    bias_cols[v] = t
                return bias_cols[v][:]

            wm = cst.tile([128, KT, L], fp16, tag="wm")
            xT_h = cst.tile([128, KT, BS], fp16, tag="xTh")
            muT_h = cst.tile([128, KT, F], fp16, tag="muTh")
            eye_t = cst.tile([128, 128], fp16, tag="eye")
            nc.sync.dma_start(eye_t[:], eye_d[:])

            # xT: load f32 staging per k-tile, cast to fp16
            for k in range(KT):
                st = stg.tile([128, 1024], f32, tag="stg")
                nc.sync.dma_start(st[:, 0:BS], xT_d[k * 128:(k + 1) * 128, :])
                nc.vector.tensor_copy(xT_h[:, k, :], st[:, 0:BS])
            # muT
            for k in range(KT):
                st = stg.tile([128, 1024], f32, tag="stg")
                nc.sync.dma_start(st[:, 0:F], muT_d[k * 128:(k + 1) * 128, :])
                nc.vector.tensor_copy(muT_h[:, k, :], st[:, 0:F])

            # m2 = sum(mu^2) per formula  ->  DRAM scratch -> row [1, F]
            m2col = cst.tile([128, F // 128], f32, tag="m2col")
            for t in range(F // 128):
                st = stg.tile([128, 1024], f32, tag="stg")
                nc.sync.dma_start(st[:, 0:D], mun_d[t * 128:(t + 1) * 128, :])
                sq = stg.tile([128, 1024], f32, tag="stg")
                nc.vector.tensor_mul(sq[:, 0:D], st[:, 0:D], st[:, 0:D])
                nc.vector.reduce_sum(m2col[:, t:t + 1], sq[:, 0:D],
                                     axis=AX.XYZW)
                nc.sync.dma_start(scr_d[t * 128:(t + 1) * 128],
                                  m2col[:, t:t + 1])
            m2row = cst.tile([1, F], f32, tag="m2row")
            nc.sync.dma_start(m2row[:], scr_d[None, :])

            sigrow = cst.tile([1, F], f32, tag="sigrow")
            nc.sync.dma_start(sigrow[:], sig_d[None, :])
            s2row = cst.tile([1, F], f32, tag="s2row")
            nc.vector.tensor_mul(s2row[:], sigrow[:], sigrow[:])
            arow = cst.tile([1, F], f32, tag="arow")
            nc.vector.reciprocal(arow[:], s2row[:])
            # beta = -0.5*m2 + ln(T)*sigma^2   (so that a*(G+beta-0.5sq)
            #   = a*(G-0.5sq-0.5m2) + ln T)
            t1 = cst.tile([1, F], f32, tag="t1row")
            nc.vector.tensor_scalar_mul(t1[:], m2row[:], -0.5)
            t2 = cst.tile([1, F], f32, tag="t2row")
            nc.vector.tensor_scalar_mul(t2[:], s2row[:], LN_T)
            brow = cst.tile([1, F], f32, tag="brow")
            nc.vector.tensor_add(brow[:], t1[:], t2[:])
            # hi/lo fp16 splits of beta and a
            bhi = cst.tile([1, F], fp16, tag="bhi")
            nc.vector.tensor_copy(bhi[:], brow[:])
            blo32 = cst.tile([1, F], f32, tag="blo32")
            nc.vector.tensor_sub(blo32[:], brow[:], bhi[:])
            blo = cst.tile([1, F], fp16, tag="blo")
            nc.vector.tensor_copy(blo[:], blo32[:])
            ahi = cst.tile([1, F], fp16, tag="ahi")
            nc.vector.tensor_copy(ahi[:], arow[:])
            alo32 = cst.tile([1, F], f32, tag="alo32")
            nc.vector.tensor_sub(alo32[:], arow[:], ahi[:])
            alo = cst.tile([1, F], fp16, tag="alo")
            nc.vector.tensor_copy(alo[:], alo32[:])
            # beta2 / a2: [2, F] fp16 via tiny DMA through scratch is
            # avoided — use two K=1 matmuls instead (ones row x row).
            ones1 = cst.tile([1, 128], fp16, tag="ones1")
            nc.vector.memset(ones1[:], 1.0)

            # A_bc = broadcast of a (1/sigma^2) to [128, F] via 2 rank-1 mms
            ps_bc = psp.tile([128, 2048], f32, tag="ps")
            nc.tensor.matmul(ps_bc[:, 0:F], ones1[:], ahi[:],
                             start=True, stop=False)
            nc.tensor.matmul(ps_bc[:, 0:F], ones1[:], alo[:],
                             start=False, stop=True)
            a_bc = cst.tile([128, F], f32, tag="a_bc")
            nc.vector.tensor_copy(a_bc[:], ps_bc[:, 0:F])

            # ---------- loc branch: G2 psums + Gram diag ----------
            sq_all = cst.tile([128, NBT], f32, tag="sq_all")
            sqh_all = cst.tile([128, NBT], f32, tag="sqh_all")
            z_all = tail.tile([128, NBT, F], fp16, tag="z_all")
            for b in range(NBT):
                bs = slice(b * 128, (b + 1) * 128)
                # Gram diag for ||x||^2
                ps_g = psp.tile([128, 2048], f32, tag="ps")
                for k in range(KT):
                    nc.tensor.matmul(ps_g[:, 0:128], xT_h[:, k, bs],
                                     xT_h[:, k, bs],
                                     start=(k == 0), stop=(k == KT - 1))
                gd = stg.tile([128, 1024], f32, tag="stg")
                nc.vector.tensor_mul(gd[:, 0:128], ps_g[:, 0:128], eye_t[:])
                nc.vector.reduce_sum(sq_all[:, b:b + 1], gd[:, 0:128],
                                     axis=AX.XYZW)
                nc.vector.tensor_scalar_mul(sqh_all[:, b:b + 1],
                                            sq_all[:, b:b + 1], 0.5)
                # G2 + rank-1 beta
                ps_G = psp.tile([128, 2048], f32, tag="ps")
                for k in range(KT):
                    nc.tensor.matmul(ps_G[:, 0:F], xT_h[:, k, bs],
                                     muT_h[:, k, :],
                                     start=(k == 0), stop=False)
                nc.tensor.matmul(ps_G[:, 0:F], ones1[:], bhi[:],
                                 start=False, stop=False)
                nc.tensor.matmul(ps_G[:, 0:F], ones1[:], blo[:],
                                 start=False, stop=True)
                # z = a * (G + beta - 0.5*||x||^2)   [-> T*exp(w) after Exp]
                nc.vector.scalar_tensor_tensor(
                    z_all[:, b, :], ps_G[:, 0:F], sqh_all[:, b:b + 1],
                    a_bc[:], op0=ALU.subtract, op1=ALU.mult)

            # ---------- Wm build ----------
            NCH = 12  # one 896-wide plane per (k, chunk) build
            for k in range(KT):
                for ch in range(NCH):
                    cs = slice(ch * CW, (ch + 1) * CW)
                    wst = stg.tile([128, 1024], f32, tag="stg")
                    nc.sync.dma_start(wst[:, 0:CW],
                                      wp_d[k * 128:(k + 1) * 128, cs])
                    mst = stg.tile([128, 1024], f32, tag="stg")
                    nc.sync.dma_start(mst[:, 0:CW],
                                      mp_d[k * 128:(k + 1) * 128, cs])
                    nc.vector.tensor_mul(wm[:, k, cs], wst[:, 0:CW],
                                         mst[:, 0:CW])

            # ---------- main loop: literals + conj (two b-halves) ----------
            HB = NBT // 2
            form_all = tail.tile([128, NBT, F], f32, tag="form_all")
            for h in range(2):
                conj_h = tail.tile([128, HB, C], fp16, tag="conj_h")
                for bl in range(HB):
                    b = h * HB + bl
                    bs = slice(b * 128, (b + 1) * 128)
                    lit = wrk.tile([128, L], fp16, tag="lit")
                    for c2 in range(PLANES // 2):   # 2-plane psum chunks
                        ps_l = psp.tile([128, 2048], f32, tag="ps")
                        for half in range(2):
                            p0 = (2 * c2 + half) * CW
                            # bank-aligned 512+384 matmul pieces per plane
                            for (o0, w_) in ((0, 512), (512, 384)):
                                cs = slice(p0 + o0, p0 + o0 + w_)
                                po = half * 1024 + o0
                                for k in range(KT):
                                    nc.tensor.matmul(
                                        ps_l[:, po:po + w_],
                                        xT_h[:, k, bs], wm[:, k, cs],
                                        start=(k == 0), stop=(k == KT - 1))
                        # tanh-evict both planes in one ACT op (3D AP)
                        pv = ps_l[:].rearrange("p (h w) -> p h w", h=2)
                        lv = lit[:].rearrange("p (i c) -> p i c", c=CW)
                        nc.scalar.activation(lv[:, 2 * c2:2 * c2 + 2, :],
                                             pv[:, :, 0:CW], ACTF.Tanh)
                    # AND stage: phase-plane adds (fp16, tree) + tanh
                    lp = lit[:].rearrange("p (i c) -> p i c", c=CW)
                    cA = conj_h[:, bl, 0:CW]
                    cB = conj_h[:, bl, CW:2 * CW]
                    cC = conj_h[:, bl, 2 * CW:3 * CW]
                    tA = ppool.tile([128, CW], fp16, tag="ppA")
                    tB = ppool.tile([128, CW], fp16, tag="ppB")
                    nc.vector.tensor_add(cA, lp[:, 0, :], lp[:, 1, :])
                    nc.vector.tensor_add(tA[:], lp[:, 2, :], lp[:, 3, :])
                    nc.vector.tensor_add(tB[:], lp[:, 4, :], lp[:, 5, :])
                    nc.vector.tensor_add(cB, tA[:], tB[:])
                    tC = ppool.tile([128, CW], fp16, tag="ppA")
                    tD = ppool.tile([128, CW], fp16, tag="ppB")
                    nc.vector.tensor_add(tC[:], lp[:, 6, :], lp[:, 7, :])
                    nc.vector.tensor_add(tD[:], lp[:, 8, :], lp[:, 9, :])
                    nc.vector.tensor_add(tC[:], tC[:], tD[:])
                    nc.vector.tensor_add(tD[:], lp[:, 10, :], lp[:, 11, :])
                    nc.vector.tensor_add(cC, tC[:], tD[:])
                    # conj = tanh(sum - d + 1.5), bias constant per class
                    for ci, d in enumerate(DEPTHS):
                        sl = conj_h[:, bl, ci * CW:(ci + 1) * CW]
                        nc.scalar.activation(sl, sl, ACTF.Tanh,
                                             bias=bias_col(1.5 - d))

                # ---------- OR stage for this half ----------
                bsl = slice(h * HB, (h + 1) * HB)
                for gi, (f0, nf, cpf_g) in enumerate(groups):
                    m = cpf_g // 3
                    parts = []
                    for ci in range(3):
                        k0 = ci * CW + gk0[gi]
                        sl = [conj_h[:, :, k0 + j * nf:k0 + (j + 1) * nf]
                              for j in range(m)]
                        acc = ppool.tile([128, HB, 64], fp16, tag=f"fp{ci}")
                        accv = acc[:, :, 0:nf]
                        if m == 2:
                            nc.vector.tensor_add(accv, sl[0], sl[1])
                        elif m == 3:
                            nc.vector.tensor_add(accv, sl[0], sl[1])
                            nc.vector.tensor_add(accv, accv, sl[2])
                        elif m == 4:
                            tmp = ppool.tile([128, HB, 64], fp16, tag="fpt")
                            tmpv = tmp[:, :, 0:nf]
                            nc.vector.tensor_add(accv, sl[0], sl[1])
                            nc.vector.tensor_add(tmpv, sl[2], sl[3])
                            nc.vector.tensor_add(accv, accv, tmpv)
                        elif m == 5:
                            tmp = ppool.tile([128, HB, 64], fp16, tag="fpt")
                            tmpv = tmp[:, :, 0:nf]
                            nc.vector.tensor_add(accv, sl[0], sl[1])
                            nc.vector.tensor_add(tmpv, sl[2], sl[3])
                            nc.vector.tensor_add(accv, accv, tmpv)
                            nc.vector.tensor_add(accv, accv, sl[4])
                        else:
                            raise AssertionError(f"unsupported cpf {cpf_g}")
                        parts.append(accv)
                    fv = form_all[:, bsl, f0:f0 + nf]
                    tmpf = ppool.tile([128, HB, 64], f32, tag="fpf32")
                    tmpfv = tmpf[:, :, 0:nf]
                    nc.vector.tensor_add(tmpfv, parts[0], parts[1])
                    nc.vector.tensor_add(fv, tmpfv, parts[2])
                    # dnnf = tanh(form + cpf - 1.5)
                    nc.scalar.activation(fv, fv, ACTF.Tanh,
                                         bias=bias_col(cpf_g - 1.5))

            # ---------- softmax tail (in place) ----------
            e_t = tail.tile([128, NBT, F], fp16, tag="e_t")
            nc.scalar.activation(z_all[:], z_all[:], ACTF.Exp)   # z = T e^w
            nc.scalar.activation(e_t[:], z_all[:], ACTF.Exp)     # e^z
            s_t = tail.tile([128, NBT], f32, tag="s_t")
            nc.vector.reduce_sum(s_t[:], e_t[:], axis=AX.X)
            r_t = tail.tile([128, NBT], f32, tag="r_t")
            nc.vector.reciprocal(r_t[:], s_t[:])
            nc.vector.tensor_mul(form_all[:], form_all[:], e_t[:])
            for b in range(NBT):
                nc.vector.tensor_scalar_mul(form_all[:, b, :],
                                            form_all[:, b, :],
                                            r_t[:, b:b + 1])
            nc.sync.dma_start(out_d.rearrange("(b p) f -> p b f", p=128),
                              form_all[:])

    nc.compile()
    _PROGRAM_CACHE[key] = nc
    return nc


def kernel(x, weight, learnable_binary_mask, bias, mu, sigma,
           lit2conj, conj2form):
    x = np.asarray(x, np.float32)
    weight = np.asarray(weight, np.float32)
    mask = np.asarray(learnable_binary_mask, np.float32)
    bias = np.asarray(bias, np.float32)
    mu = np.asarray(mu, np.float32)
    sigma = np.asarray(sigma, np.float32)
    lit2conj = np.asarray(lit2conj, np.int64)
    conj2form = np.asarray(conj2form, np.int64)

    groups, cpf, cstart = _derive_structure(lit2conj, conj2form)
    inv, gk0 = _build_permutation(lit2conj, conj2form, groups, cpf, cstart)
    bias_zero = bool(np.all(bias == 0))

    nc = _build_program(groups, gk0, bias_zero)

    wp = np.ascontiguousarray(weight[:, inv])
    mp = np.ascontiguousarray(mask[:, inv])
    muT = np.ascontiguousarray(mu.T)
    eye = np.eye(128, dtype=np.float16)

    in_maps = []
    for i in range(NCORES):
        xs = x[i * BS:(i + 1) * BS]
        in_maps.append({
            "xT": np.ascontiguousarray(xs.T),
            "wp": wp, "mp": mp, "muT": muT, "mun": mu,
            "sig": sigma, "eye": eye,
        })

    res = bass_utils.run_bass_kernel_spmd(nc, in_maps,
                                          core_ids=list(range(NCORES)))
    out = np.concatenate([res.results[i]["out"] for i in range(NCORES)],
                         axis=0)
    return out.astype(np.float32)
